# revision 28
# baseline (speedup 1.0000x reference)
"""Multi-head self-attention (B=1, S=4096, D=512, H=8) on 8 trn2 NeuronCores.

Sharding: one head per core (head/tensor parallel). Each core computes its
head's Q/K/V projections from the full (transposed, bf16) query, runs
attention streaming over key chunks (softmax denominator via a ones-column
augmented V^T), applies its slice of out_proj fused with softmax
normalization, and writes an unnormalized partial [S, D] output. Host sums
the 8 partials and adds out_proj bias.

Perf notes (HW-measured):
- The PE clock-gate (HAM) throttles to 1.2GHz unless the PE is busy a full
  ~3.4us activity window; back-to-back matmuls sustain 2.4GHz. A junk
  warm-up matmul stream covers the input-DMA latency and flips the clock
  early; the schedule then keeps PE gaps small.
- Matmul streaming is 1 column/cycle for f32r and bf16 alike, so scores use
  f32r (better accuracy, same speed); qt/weights/P/V^T/out^T are bf16 to
  halve DMA+SBUF traffic.
- The softmax exp runs on the ACT engine (~15.3us/group of 512 queries) and
  is rate-matched with the PE (~14.5us/group); out_proj matmuls of group
  g-1 are interleaved into group g's stream to fill PE slack.
"""

import sys

sys.path.insert(0, "/opt/trn_rl_repo")

import numpy as np

EMBED = 512
HEADS = 8
HD = 64          # head dim
S = 4096         # sequence length
P = 128          # partitions
NSK = S // P     # 32 key chunks of 128
QG = 512         # query group width (matmul free dim)
NQG = S // QG    # 8 query groups
NDC = EMBED // P # 4 contraction chunks for projections
SCALE = HD ** -0.5
EXP_BATCH = 3    # key chunks per exp batch (PSUM banks per score slot)
N_JUNK = 14      # warm-up matmuls to flip the PE clock-gate early
JW = 256         # junk filler matmul width in stage C (0 disables)
A_LAG = 1        # batches the P.V matmul trails the scores matmul by

_compiled = {}


def _build(n_cores=8):
    import concourse.bacc as bacc
    import concourse.mybir as mybir
    import concourse.tile as tile

    f32 = mybir.dt.float32
    bf16 = mybir.dt.bfloat16

    nc = bacc.Bacc("TRN2", target_bir_lowering=False, debug=False,
                   num_devices=n_cores)

    qt = nc.dram_tensor("qt", [EMBED, S], bf16, kind="ExternalInput")
    wq = nc.dram_tensor("wq", [EMBED, HD], bf16, kind="ExternalInput")
    wk = nc.dram_tensor("wk", [EMBED, HD], bf16, kind="ExternalInput")
    wv = nc.dram_tensor("wv", [EMBED, HD], bf16, kind="ExternalInput")
    wo = nc.dram_tensor("wo", [HD, EMBED], bf16, kind="ExternalInput")
    bq = nc.dram_tensor("bq", [HD, 1], f32, kind="ExternalInput")
    bk = nc.dram_tensor("bk", [HD, 1], f32, kind="ExternalInput")
    bv = nc.dram_tensor("bv", [P, HD], f32, kind="ExternalInput")
    out_p = nc.dram_tensor("out_p", [S, EMBED], f32, kind="ExternalOutput")

    with tile.TileContext(nc) as tc:
        _emit(tc, nc, mybir, qt, wq, wk, wv, wo, bq, bk, bv, out_p)

    nc.compile()
    return nc


def _emit(tc, nc, mybir, qt, wq, wk, wv, wo, bq, bk, bv, out_p):
    from contextlib import ExitStack

    f32 = mybir.dt.float32
    f32r = mybir.dt.float32r
    bf16 = mybir.dt.bfloat16
    Exp = mybir.ActivationFunctionType.Exp

    with ExitStack() as ctx:
        singles = ctx.enter_context(tc.tile_pool(name="singles", bufs=1))

        # --- warm up the ACT exp table while DMAs run ---
        warm = singles.tile([1, 1], f32)
        nc.vector.memset(warm, 0.0)
        warm2 = singles.tile([1, 1], f32)
        nc.scalar.activation(warm2, warm, Exp)

        # --- junk tile for PE clock warm-up ---
        junk = singles.tile([P, QG], bf16)
        nc.vector.memset(junk, 0.0)

        # --- weights + biases (small, first) ---
        wq_sb = singles.tile([P, NDC, HD], bf16)
        wk_sb = singles.tile([P, NDC, HD], bf16)
        wv_sb = singles.tile([P, NDC, HD], bf16)
        for c in range(NDC):
            nc.gpsimd.dma_start(out=wq_sb[:, c, :], in_=wq[c * P:(c + 1) * P, :])
            nc.gpsimd.dma_start(out=wk_sb[:, c, :], in_=wk[c * P:(c + 1) * P, :])
            nc.gpsimd.dma_start(out=wv_sb[:, c, :], in_=wv[c * P:(c + 1) * P, :])
        wo_sb = singles.tile([HD, EMBED], bf16)
        nc.gpsimd.dma_start(out=wo_sb, in_=wo[:, :])
        bq_sb = singles.tile([HD, 1], f32)
        nc.gpsimd.dma_start(out=bq_sb, in_=bq[:, :])
        bk_sb = singles.tile([HD, 1], f32)
        nc.gpsimd.dma_start(out=bk_sb, in_=bk[:, :])
        bv_sb = singles.tile([P, HD], f32)
        nc.gpsimd.dma_start(out=bv_sb, in_=bv[:, :])

        # --- qt load: 2-group slices, first groups prioritized ---
        qt_sb = []
        for c in range(NDC):
            t = singles.tile([P, S], bf16, tag=f"qt{c}")
            qt_sb.append(t)
        dma_engs = [nc.sync, nc.scalar, nc.gpsimd]
        di = 0
        for g2 in range(NQG // 2):
            sl = slice(g2 * 2 * QG, (g2 + 1) * 2 * QG)
            for c in range(NDC):
                dma_engs[di % 3].dma_start(
                    out=qt_sb[c][:, sl], in_=qt[c * P:(c + 1) * P, sl])
                di += 1

        # persistent activations
        q_sb = singles.tile([HD, S], f32r)        # Q^T per head
        k_sb = singles.tile([HD, S], f32r)        # K^T per head
        vt_sb = singles.tile([P, NSK, HD + 1], bf16)  # V^T chunks + ones col
        ot_sb = singles.tile([HD, S], bf16)       # unnormalized attn out^T
        den_row = singles.tile([1, S], f32)       # denominators, row layout
        den_all = singles.tile([P, NSK], f32)     # denominators, [s%128, blk]
        recip_all = singles.tile([P, NSK], f32)   # 1/denominator

        nc.vector.memset(vt_sb[:, :, HD:HD + 1], 1.0)

        # --- PE clock warm-up + stage B: projections ---
        with ExitStack() as bctx:
            jp = bctx.enter_context(
                tc.tile_pool(name="jp", bufs=1, space="PSUM"))
            pqk = bctx.enter_context(
                tc.tile_pool(name="pqk", bufs=2, space="PSUM"))
            pvp = bctx.enter_context(
                tc.tile_pool(name="pvp", bufs=2, space="PSUM"))

            j_ps = jp.tile([P, QG], f32)

            def emit_junk(n, width=QG):
                for _ in range(n):
                    nc.tensor.matmul(j_ps[:, 0:width], junk[:, 0:P],
                                     junk[:, 0:width], start=True, stop=True)

            emit_junk(N_JUNK)

            for g in range(NQG):
                sl = slice(g * QG, (g + 1) * QG)
                acc_q = pqk.tile([HD, QG], f32, tag="pj")
                for c in range(NDC):
                    nc.tensor.matmul(acc_q, wq_sb[:, c, :], qt_sb[c][:, sl],
                                     start=(c == 0), stop=(c == NDC - 1))
                nc.vector.tensor_scalar_add(q_sb[:, sl], acc_q, bq_sb)
                acc_k = pqk.tile([HD, QG], f32, tag="pj")
                for c in range(NDC):
                    nc.tensor.matmul(acc_k, wk_sb[:, c, :], qt_sb[c][:, sl],
                                     start=(c == 0), stop=(c == NDC - 1))
                nc.vector.tensor_scalar_add(k_sb[:, sl], acc_k, bk_sb)
                if g < 6:
                    # absorb qt DMA jitter without idling the PE array
                    emit_junk(2)

            for s in range(8):
                ssl = slice(s * P, (s + 1) * P)
                acc_v = pvp.tile([P, HD], f32, tag="pv")
                for c in range(NDC):
                    nc.tensor.matmul(acc_v, qt_sb[c][:, ssl], wv_sb[:, c, :],
                                     start=(c == 0), stop=(c == NDC - 1))
                nc.vector.tensor_add(vt_sb[:, s, 0:HD], acc_v, bv_sb)

        # --- stage C: attention + interleaved stage D (out_proj) ---
        # A single flat batch stream across all query groups: the scores
        # matmuls (S), exp activations, and P.V matmuls (A, lagging by
        # A_LAG batches) flow continuously so the ACT engine never idles
        # at group boundaries. Stage-D out_proj blocks of the previous
        # group and junk matmuls fill the PE's slack (the ACT engine is
        # the rate limiter), keeping the PE clock-gate at full speed.
        with ExitStack() as cctx:
            s_pool = cctx.enter_context(
                tc.tile_pool(name="s_pool", bufs=1, space="PSUM"))
            acc_pool = cctx.enter_context(
                tc.tile_pool(name="acc_pool", bufs=1, space="PSUM"))
            o_pool = cctx.enter_context(
                tc.tile_pool(name="o_pool", bufs=1, space="PSUM"))
            jc_psum = cctx.enter_context(
                tc.tile_pool(name="jc_psum", bufs=1, space="PSUM"))
            p_pool = cctx.enter_context(tc.tile_pool(name="p_pool", bufs=5))
            osb_pool = cctx.enter_context(tc.tile_pool(name="osb", bufs=4))
            jc_pool = cctx.enter_context(tc.tile_pool(name="jc", bufs=1))

            junk_c = jc_pool.tile([P, QG], bf16)
            nc.vector.memset(junk_c, 0.0)

            # alternating 3/2-chunk batches: the two score slots are 3+2
            # PSUM banks, leaving banks for acc, out_proj, and junk
            SZ = [3, 2] * 7
            SZ = SZ[:12] + [2]           # 6x3 + 7x2 = 32 chunks
            CS = [0]
            for z in SZ:
                CS.append(CS[-1] + z)
            NB = len(SZ)                 # 13 batches per group
            NJ = NQG * NB                # total batches
            D_BATCHES = (4, 6, 8, 10)    # local batch slots carrying stage D
            out_accs = [None] * NQG
            p_tiles = [None] * NJ

            j_ps2 = jc_psum.tile([P, QG], f32)

            def emit_junk_c(n=1):
                for _ in range(n):
                    nc.tensor.matmul(j_ps2[:, 0:JW], junk_c[:, 0:P],
                                     junk_c[:, 0:JW], start=True, stop=True)

            def emit_d_block(g, i):
                # out_proj for s-block i of group g, normalization fused
                t = g * (QG // P) + i
                tsl = slice(t * P, (t + 1) * P)
                o_ps = o_pool.tile([P, EMBED], f32, tag="ops")
                nc.tensor.matmul(o_ps, ot_sb[:, tsl], wo_sb,
                                 start=True, stop=True)
                o_sb = osb_pool.tile([P, EMBED], f32, tag="o")
                nc.vector.tensor_scalar_mul(o_sb, o_ps, recip_all[:, t:t + 1])
                nc.sync.dma_start(out=out_p[tsl, :], in_=o_sb)

            def emit_s(j):
                g, b = divmod(j, NB)
                if b == 0:
                    out_accs[g] = acc_pool.tile([HD + 1, QG], f32, tag="acc",
                                                name=f"out_acc{g}")
                gsl = slice(g * QG, (g + 1) * QG)
                nb = SZ[b]
                chunks = range(CS[b], CS[b] + nb)
                s_ps = s_pool.tile([P, nb * QG], f32,
                                   tag=("spsA" if b % 2 == 0 else "spsB"),
                                   name=f"sps{j}")
                for i, s in enumerate(chunks):
                    nc.tensor.matmul(
                        s_ps[:, i * QG:(i + 1) * QG],
                        k_sb[:, s * P:(s + 1) * P], q_sb[:, gsl],
                        start=True, stop=True)
                p_sb = p_pool.tile([P, 3 * QG], bf16, tag="p")
                p_tiles[j] = p_sb
                nc.scalar.activation(p_sb[:, :nb * QG], s_ps, Exp,
                                     scale=SCALE)

            def emit_a(j):
                g, b = divmod(j, NB)
                p_sb = p_tiles[j]
                for i, s in enumerate(range(CS[b], CS[b] + SZ[b])):
                    nc.tensor.matmul(
                        out_accs[g], vt_sb[:, s, :],
                        p_sb[:, i * QG:(i + 1) * QG],
                        start=(s == 0), stop=(s == NSK - 1))
                if b == NB - 1:
                    emit_group_end(g)

            def emit_group_end(g):
                # evict numerators (bf16 for out_proj) and denominators,
                # then transpose denominators via SBUF-SBUF DMA and invert
                gsl = slice(g * QG, (g + 1) * QG)
                out_acc = out_accs[g]
                nc.vector.tensor_copy(ot_sb[:, gsl], out_acc[0:HD, :])
                nc.vector.tensor_copy(den_row[:, gsl], out_acc[HD:HD + 1, :])
                for i in range(QG // P):
                    j = g * (QG // P) + i
                    nc.gpsimd.dma_start(
                        out=den_all[:, j:j + 1],
                        in_=den_row[0:1, j * P:(j + 1) * P])
                j0 = g * (QG // P)
                nc.vector.reciprocal(recip_all[:, j0:j0 + QG // P],
                                     den_all[:, j0:j0 + QG // P])

            def emit_v_block(s):
                # just-in-time V^T chunk projection, borrowing an o_pool
                # ring slot for the PSUM accumulator
                ssl = slice(s * P, (s + 1) * P)
                o_t = o_pool.tile([P, EMBED], f32, tag="ops",
                                  name=f"vacc{s}")
                acc_v = o_t[:, 0:HD]
                for c in range(NDC):
                    nc.tensor.matmul(acc_v, qt_sb[c][:, ssl], wv_sb[:, c, :],
                                     start=(c == 0), stop=(c == NDC - 1))
                nc.vector.tensor_add(vt_sb[:, s, 0:HD], acc_v, bv_sb)

            def emit_filler(j):
                g, b = divmod(j, NB)
                if g == 0:
                    # group 0: stream the remaining V^T projections two
                    # batches ahead of their consumption by emit_a
                    n_v = 0
                    for s in (3 * b + 6, 3 * b + 7, 3 * b + 8):
                        if s < NSK:
                            emit_v_block(s)
                            n_v += 1
                    if n_v == 0:
                        emit_junk_c(3)
                elif b in D_BATCHES:
                    emit_d_block(g - 1, D_BATCHES.index(b))
                    emit_junk_c(2)
                else:
                    emit_junk_c(3)

            v_next = [8]
            for j in range(NJ + A_LAG):
                if j < NJ:
                    emit_s(j)
                if j >= A_LAG:
                    emit_a(j - A_LAG)
                if j < NJ:
                    emit_filler(j)
            for i in range(QG // P):
                emit_d_block(NQG - 1, i)


def _in_maps(query, in_proj_weight, in_proj_bias, out_proj_weight):
    import ml_dtypes
    bf16 = ml_dtypes.bfloat16
    q2d = np.asarray(query, dtype=np.float32).reshape(S, EMBED)
    qt = np.ascontiguousarray(q2d.T).astype(bf16)
    w = np.asarray(in_proj_weight, dtype=np.float32)
    b = np.asarray(in_proj_bias, dtype=np.float32)
    wout = np.asarray(out_proj_weight, dtype=np.float32)
    maps = []
    for h in range(HEADS):
        hs = slice(h * HD, (h + 1) * HD)
        maps.append({
            "qt": qt,
            "wq": np.ascontiguousarray(w[hs, :].T).astype(bf16),
            "wk": np.ascontiguousarray(
                w[EMBED + h * HD:EMBED + (h + 1) * HD, :].T).astype(bf16),
            "wv": np.ascontiguousarray(
                w[2 * EMBED + h * HD:2 * EMBED + (h + 1) * HD, :].T).astype(bf16),
            "wo": np.ascontiguousarray(wout[:, hs].T).astype(bf16),
            "bq": np.ascontiguousarray(b[hs].reshape(HD, 1)),
            "bk": np.ascontiguousarray(
                b[EMBED + h * HD:EMBED + (h + 1) * HD].reshape(HD, 1)),
            "bv": np.ascontiguousarray(np.broadcast_to(
                b[2 * EMBED + h * HD:2 * EMBED + (h + 1) * HD], (P, HD))),
        })
    return maps


def get_nc():
    if "nc" not in _compiled:
        _compiled["nc"] = _build()
    return _compiled["nc"]


def kernel(query, in_proj_weight, in_proj_bias, out_proj_weight, out_proj_bias):
    from concourse.bass_utils import run_bass_kernel_spmd

    nc = get_nc()
    maps = _in_maps(query, in_proj_weight, in_proj_bias, out_proj_weight)
    res = run_bass_kernel_spmd(nc, maps, core_ids=list(range(HEADS)))
    acc = np.zeros((S, EMBED), dtype=np.float32)
    for h in range(HEADS):
        acc += res.results[h]["out_p"]
    acc += np.asarray(out_proj_bias, dtype=np.float32)[None, :]
    return acc.reshape(np.asarray(query).shape).astype(np.float32)


# revision 30
# speedup vs baseline: 1.5284x; 1.5284x over previous
"""Multi-head self-attention (B=1, S=4096, D=512, H=8) on 8 trn2 NeuronCores.

Sharding: one head per core (head/tensor parallel). Each core computes its
head's Q/K/V projections from the full (transposed, bf16) query, runs
attention streaming over key chunks (softmax denominator via a ones-column
augmented V^T), applies its slice of out_proj fused with softmax
normalization, and writes an unnormalized partial [S, D] output. Host sums
the 8 partials and adds out_proj bias.

Perf notes (HW-measured):
- The PE clock-gate (HAM) throttles to 1.2GHz unless the PE is busy a full
  ~3.4us activity window; back-to-back matmuls sustain 2.4GHz. A junk
  warm-up matmul stream covers the input-DMA latency and flips the clock
  early; the schedule then keeps PE gaps small.
- Matmul streaming is 1 column/cycle for f32r and bf16 alike, so scores use
  f32r (better accuracy, same speed); qt/weights/P/V^T/out^T are bf16 to
  halve DMA+SBUF traffic.
- The softmax exp runs on the ACT engine (~15.3us/group of 512 queries) and
  is rate-matched with the PE (~14.5us/group); out_proj matmuls of group
  g-1 are interleaved into group g's stream to fill PE slack.
"""

import sys

sys.path.insert(0, "/opt/trn_rl_repo")

import numpy as np

EMBED = 512
HEADS = 8
HD = 64          # head dim
S = 4096         # sequence length
P = 128          # partitions
NSK = S // P     # 32 key chunks of 128
QG = 512         # query group width (matmul free dim)
NQG = S // QG    # 8 query groups
NDC = EMBED // P # 4 contraction chunks for projections
SCALE = HD ** -0.5
EXP_BATCH = 2    # key chunks per exp batch (PSUM banks per score slot)
N_JUNK = 18      # warm-up matmuls to flip the PE clock-gate early
JW = 256         # junk filler matmul width in stage C (0 disables)
A_LAG = 1        # batches the P.V matmul trails the scores matmul by

_compiled = {}


def _build(n_cores=8):
    import concourse.bacc as bacc
    import concourse.mybir as mybir
    import concourse.tile as tile

    f32 = mybir.dt.float32
    bf16 = mybir.dt.bfloat16

    nc = bacc.Bacc("TRN2", target_bir_lowering=False, debug=False,
                   num_devices=n_cores)

    qt = nc.dram_tensor("qt", [EMBED, S], bf16, kind="ExternalInput")
    wq = nc.dram_tensor("wq", [EMBED, HD], bf16, kind="ExternalInput")
    wk = nc.dram_tensor("wk", [EMBED, HD], bf16, kind="ExternalInput")
    wv = nc.dram_tensor("wv", [EMBED, HD], bf16, kind="ExternalInput")
    wo = nc.dram_tensor("wo", [HD, EMBED], bf16, kind="ExternalInput")
    bq = nc.dram_tensor("bq", [HD, 1], f32, kind="ExternalInput")
    bk = nc.dram_tensor("bk", [HD, 1], f32, kind="ExternalInput")
    bv = nc.dram_tensor("bv", [P, HD], f32, kind="ExternalInput")
    out_p = nc.dram_tensor("out_p", [S, EMBED], f32, kind="ExternalOutput")

    with tile.TileContext(nc) as tc:
        _emit(tc, nc, mybir, qt, wq, wk, wv, wo, bq, bk, bv, out_p)

    nc.compile()
    return nc


def _emit(tc, nc, mybir, qt, wq, wk, wv, wo, bq, bk, bv, out_p):
    from contextlib import ExitStack

    f32 = mybir.dt.float32
    f32r = mybir.dt.float32r
    bf16 = mybir.dt.bfloat16
    Exp = mybir.ActivationFunctionType.Exp

    with ExitStack() as ctx:
        singles = ctx.enter_context(tc.tile_pool(name="singles", bufs=1))

        # --- warm up the ACT exp table while DMAs run ---
        warm = singles.tile([1, 1], f32)
        nc.vector.memset(warm, 0.0)
        warm2 = singles.tile([1, 1], f32)
        nc.scalar.activation(warm2, warm, Exp)

        # --- junk tile for PE clock warm-up ---
        junk = singles.tile([P, QG], bf16)
        nc.vector.memset(junk, 0.0)

        # --- weights + biases (small, first) ---
        wq_sb = singles.tile([P, NDC, HD], bf16)
        wk_sb = singles.tile([P, NDC, HD], bf16)
        wv_sb = singles.tile([P, NDC, HD], bf16)
        for c in range(NDC):
            nc.gpsimd.dma_start(out=wq_sb[:, c, :], in_=wq[c * P:(c + 1) * P, :])
            nc.gpsimd.dma_start(out=wk_sb[:, c, :], in_=wk[c * P:(c + 1) * P, :])
            nc.gpsimd.dma_start(out=wv_sb[:, c, :], in_=wv[c * P:(c + 1) * P, :])
        wo_sb = singles.tile([HD, EMBED], bf16)
        nc.gpsimd.dma_start(out=wo_sb, in_=wo[:, :])
        bq_sb = singles.tile([HD, 1], f32)
        nc.gpsimd.dma_start(out=bq_sb, in_=bq[:, :])
        bk_sb = singles.tile([HD, 1], f32)
        nc.gpsimd.dma_start(out=bk_sb, in_=bk[:, :])
        bv_sb = singles.tile([P, HD], f32)
        nc.gpsimd.dma_start(out=bv_sb, in_=bv[:, :])

        # --- qt load: 2-group slices, first groups prioritized ---
        qt_sb = []
        for c in range(NDC):
            t = singles.tile([P, S], bf16, tag=f"qt{c}")
            qt_sb.append(t)
        dma_engs = [nc.sync, nc.scalar, nc.gpsimd]
        di = 0
        for g2 in range(NQG // 2):
            sl = slice(g2 * 2 * QG, (g2 + 1) * 2 * QG)
            for c in range(NDC):
                dma_engs[di % 3].dma_start(
                    out=qt_sb[c][:, sl], in_=qt[c * P:(c + 1) * P, sl])
                di += 1

        # persistent activations
        q_sb = singles.tile([HD, S], f32r)        # Q^T per head
        k_sb = singles.tile([HD, S], f32r)        # K^T per head
        vt_sb = singles.tile([P, NSK, HD + 1], bf16)  # V^T chunks + ones col
        ot_sb = singles.tile([HD, S], bf16)       # unnormalized attn out^T
        den_row = singles.tile([1, S], f32)       # denominators, row layout
        den_all = singles.tile([P, NSK], f32)     # denominators, [s%128, blk]
        recip_all = singles.tile([P, NSK], f32)   # 1/denominator

        nc.vector.memset(vt_sb[:, :, HD:HD + 1], 1.0)

        # --- PE clock warm-up + stage B: projections ---
        with ExitStack() as bctx:
            jp = bctx.enter_context(
                tc.tile_pool(name="jp", bufs=1, space="PSUM"))
            pqk = bctx.enter_context(
                tc.tile_pool(name="pqk", bufs=2, space="PSUM"))
            pvp = bctx.enter_context(
                tc.tile_pool(name="pvp", bufs=2, space="PSUM"))

            j_ps = jp.tile([P, QG], f32)

            def emit_junk(n, width=QG):
                for _ in range(n):
                    nc.tensor.matmul(j_ps[:, 0:width], junk[:, 0:P],
                                     junk[:, 0:width], start=True, stop=True)

            emit_junk(N_JUNK)

            for g in range(NQG):
                sl = slice(g * QG, (g + 1) * QG)
                acc_q = pqk.tile([HD, QG], f32, tag="pj")
                for c in range(NDC):
                    nc.tensor.matmul(acc_q, wq_sb[:, c, :], qt_sb[c][:, sl],
                                     start=(c == 0), stop=(c == NDC - 1))
                nc.vector.tensor_scalar_add(q_sb[:, sl], acc_q, bq_sb)
                acc_k = pqk.tile([HD, QG], f32, tag="pj")
                for c in range(NDC):
                    nc.tensor.matmul(acc_k, wk_sb[:, c, :], qt_sb[c][:, sl],
                                     start=(c == 0), stop=(c == NDC - 1))
                nc.vector.tensor_scalar_add(k_sb[:, sl], acc_k, bk_sb)
                if g < 6:
                    # absorb qt DMA jitter without idling the PE array
                    emit_junk(2)

            for s in range(4):
                ssl = slice(s * P, (s + 1) * P)
                acc_v = pvp.tile([P, HD], f32, tag="pv")
                for c in range(NDC):
                    nc.tensor.matmul(acc_v, qt_sb[c][:, ssl], wv_sb[:, c, :],
                                     start=(c == 0), stop=(c == NDC - 1))
                nc.vector.tensor_add(vt_sb[:, s, 0:HD], acc_v, bv_sb)

        # --- stage C: attention + interleaved stage D (out_proj) ---
        # A single flat batch stream across all query groups: the scores
        # matmuls (S), exp activations, and P.V matmuls (A, lagging by
        # A_LAG batches) flow continuously so the ACT engine never idles
        # at group boundaries. Stage-D out_proj blocks of the previous
        # group and junk matmuls fill the PE's slack (the ACT engine is
        # the rate limiter), keeping the PE clock-gate at full speed.
        with ExitStack() as cctx:
            s_pool = cctx.enter_context(
                tc.tile_pool(name="s_pool", bufs=2, space="PSUM"))
            acc_pool = cctx.enter_context(
                tc.tile_pool(name="acc_pool", bufs=1, space="PSUM"))
            o_pool = cctx.enter_context(
                tc.tile_pool(name="o_pool", bufs=2, space="PSUM"))
            jc_psum = cctx.enter_context(
                tc.tile_pool(name="jc_psum", bufs=1, space="PSUM"))
            p_pool = cctx.enter_context(tc.tile_pool(name="p_pool", bufs=5))
            osb_pool = cctx.enter_context(tc.tile_pool(name="osb", bufs=4))
            jc_pool = cctx.enter_context(tc.tile_pool(name="jc", bufs=1))

            junk_c = jc_pool.tile([P, QG], bf16)
            nc.vector.memset(junk_c, 0.0)

            NB = -(-NSK // EXP_BATCH)    # batches per group
            NJ = NQG * NB                # total batches
            D_BATCHES = (4, 6, 8, 10)    # local batch slots carrying stage D
            out_accs = [None] * NQG
            p_tiles = [None] * NJ

            j_ps2 = jc_psum.tile([P, QG], f32)

            def emit_junk_c(n=1):
                for _ in range(n):
                    nc.tensor.matmul(j_ps2[:, 0:JW], junk_c[:, 0:P],
                                     junk_c[:, 0:JW], start=True, stop=True)

            def emit_d_block(g, i):
                # out_proj for s-block i of group g, normalization fused
                t = g * (QG // P) + i
                tsl = slice(t * P, (t + 1) * P)
                o_ps = o_pool.tile([P, EMBED], f32, tag="ops")
                nc.tensor.matmul(o_ps, ot_sb[:, tsl], wo_sb,
                                 start=True, stop=True)
                o_sb = osb_pool.tile([P, EMBED], f32, tag="o")
                nc.vector.tensor_scalar_mul(o_sb, o_ps, recip_all[:, t:t + 1])
                nc.sync.dma_start(out=out_p[tsl, :], in_=o_sb)

            def emit_s(j):
                g, b = divmod(j, NB)
                if b == 0:
                    out_accs[g] = acc_pool.tile([HD + 1, QG], f32, tag="acc",
                                                name=f"out_acc{g}")
                gsl = slice(g * QG, (g + 1) * QG)
                chunks = range(b * EXP_BATCH, min((b + 1) * EXP_BATCH, NSK))
                nb = len(chunks)
                s_ps = s_pool.tile([P, EXP_BATCH * QG], f32, tag="sps")
                for i, s in enumerate(chunks):
                    nc.tensor.matmul(
                        s_ps[:, i * QG:(i + 1) * QG],
                        k_sb[:, s * P:(s + 1) * P], q_sb[:, gsl],
                        start=True, stop=True)
                p_sb = p_pool.tile([P, EXP_BATCH * QG], bf16, tag="p")
                p_tiles[j] = p_sb
                nc.scalar.activation(p_sb[:, :nb * QG], s_ps[:, :nb * QG],
                                     Exp, scale=SCALE)

            def emit_a(j):
                g, b = divmod(j, NB)
                p_sb = p_tiles[j]
                for i, s in enumerate(range(b * EXP_BATCH,
                                            min((b + 1) * EXP_BATCH, NSK))):
                    nc.tensor.matmul(
                        out_accs[g], vt_sb[:, s, :],
                        p_sb[:, i * QG:(i + 1) * QG],
                        start=(s == 0), stop=(s == NSK - 1))
                if b == NB - 1:
                    emit_group_end(g)

            def emit_group_end(g):
                # evict numerators (bf16 for out_proj) and denominators,
                # then transpose denominators via SBUF-SBUF DMA and invert
                gsl = slice(g * QG, (g + 1) * QG)
                out_acc = out_accs[g]
                nc.vector.tensor_copy(ot_sb[:, gsl], out_acc[0:HD, :])
                nc.vector.tensor_copy(den_row[:, gsl], out_acc[HD:HD + 1, :])
                for i in range(QG // P):
                    j = g * (QG // P) + i
                    nc.gpsimd.dma_start(
                        out=den_all[:, j:j + 1],
                        in_=den_row[0:1, j * P:(j + 1) * P])
                j0 = g * (QG // P)
                nc.vector.reciprocal(recip_all[:, j0:j0 + QG // P],
                                     den_all[:, j0:j0 + QG // P])

            def emit_v_block(s):
                # just-in-time V^T chunk projection, borrowing an o_pool
                # ring slot for the PSUM accumulator
                ssl = slice(s * P, (s + 1) * P)
                o_t = o_pool.tile([P, EMBED], f32, tag="ops",
                                  name=f"vacc{s}")
                acc_v = o_t[:, 0:HD]
                for c in range(NDC):
                    nc.tensor.matmul(acc_v, qt_sb[c][:, ssl], wv_sb[:, c, :],
                                     start=(c == 0), stop=(c == NDC - 1))
                nc.vector.tensor_add(vt_sb[:, s, 0:HD], acc_v, bv_sb)

            def emit_filler(j):
                g, b = divmod(j, NB)
                if g == 0:
                    # group 0: stream the remaining V^T projections two
                    # batches ahead of their consumption by emit_a
                    for s in (2 * b + 4, 2 * b + 5):
                        if s < NSK:
                            emit_v_block(s)
                    if b >= NB - 2:
                        emit_junk_c(1)
                elif b in D_BATCHES:
                    emit_d_block(g - 1, D_BATCHES.index(b))
                else:
                    emit_junk_c(1)

            for j in range(NJ + A_LAG):
                if j < NJ:
                    emit_s(j)
                if j >= A_LAG:
                    emit_a(j - A_LAG)
                if j < NJ:
                    emit_filler(j)
            for i in range(QG // P):
                emit_d_block(NQG - 1, i)


def _in_maps(query, in_proj_weight, in_proj_bias, out_proj_weight):
    import ml_dtypes
    bf16 = ml_dtypes.bfloat16
    q2d = np.asarray(query, dtype=np.float32).reshape(S, EMBED)
    qt = np.ascontiguousarray(q2d.T).astype(bf16)
    w = np.asarray(in_proj_weight, dtype=np.float32)
    b = np.asarray(in_proj_bias, dtype=np.float32)
    wout = np.asarray(out_proj_weight, dtype=np.float32)
    maps = []
    for h in range(HEADS):
        hs = slice(h * HD, (h + 1) * HD)
        maps.append({
            "qt": qt,
            "wq": np.ascontiguousarray(w[hs, :].T).astype(bf16),
            "wk": np.ascontiguousarray(
                w[EMBED + h * HD:EMBED + (h + 1) * HD, :].T).astype(bf16),
            "wv": np.ascontiguousarray(
                w[2 * EMBED + h * HD:2 * EMBED + (h + 1) * HD, :].T).astype(bf16),
            "wo": np.ascontiguousarray(wout[:, hs].T).astype(bf16),
            "bq": np.ascontiguousarray(b[hs].reshape(HD, 1)),
            "bk": np.ascontiguousarray(
                b[EMBED + h * HD:EMBED + (h + 1) * HD].reshape(HD, 1)),
            "bv": np.ascontiguousarray(np.broadcast_to(
                b[2 * EMBED + h * HD:2 * EMBED + (h + 1) * HD], (P, HD))),
        })
    return maps


def get_nc():
    if "nc" not in _compiled:
        _compiled["nc"] = _build()
    return _compiled["nc"]


def kernel(query, in_proj_weight, in_proj_bias, out_proj_weight, out_proj_bias):
    from concourse.bass_utils import run_bass_kernel_spmd

    nc = get_nc()
    maps = _in_maps(query, in_proj_weight, in_proj_bias, out_proj_weight)
    res = run_bass_kernel_spmd(nc, maps, core_ids=list(range(HEADS)))
    acc = np.zeros((S, EMBED), dtype=np.float32)
    for h in range(HEADS):
        acc += res.results[h]["out_p"]
    acc += np.asarray(out_proj_bias, dtype=np.float32)[None, :]
    return acc.reshape(np.asarray(query).shape).astype(np.float32)
